# revision 26
# baseline (speedup 1.0000x reference)
"""LRU (Linear Recurrent Unit) single-step forward on 8 Trainium2 NeuronCores.

Math: with seq-len 1 the whole LRU step collapses algebraically to one GEMM:
    y[b,:] = W @ u[b] + bias
where
    W    = 2*C_re@diag(g)@B_re - 2*C_im@diag(g)@B_im + D          [DOUT, DIN]
    bias = 2*(C_re@(lam_re*x_re - lam_im*x_im)
              - C_im@(lam_re*x_im + lam_im*x_re))                  [DOUT]
    g = exp(gamma_log), lam = exp(-exp(nu_log)) * exp(i*exp(theta_log)).

The parameter fold (W, bias) is input-data independent, computed once on host
in float64.  The batch GEMM (99% of FLOPs and bytes) runs on the 8 NeuronCores,
data-parallel over the batch: each core computes y_shard^T = W @ u_shard^T
(+bias).

Everything on the wire is fp16 (u, W, y): quantization rel-err ~3e-4, far
inside the 2e-2 gate, and it halves HBM traffic (10.4 MB/core vs 18.9) so the
kernel is PE-streaming-bound, not DMA-bound.  The PE consumes fp16 at the same
1 elem/cell/cycle as fp32r, so the matmul stream is the roofline: 131072
PE-column cycles ~= 54.6 us warm at 2.4 GHz.

Device kernel layout (per core):
  wu0 [8, 128, 1536]  [W^T block ib | u0 block ib] packed: one DMA delivers
                      a full (stationary, moving) pair for the pipelined start
  u1 [8, 128, 1536]   u^T blocked, batch tiles 1-3
  bias [128, 8]       bias[jb*128+p] at [p, jb]
  yb [8, 128, 2048]   y^T row-blocked by jb
Schedule notes:
  - N=128 junk matmuls (stationary memset by the otherwise-idle GpSimd)
    release the HAM clock gate (1.2 -> 2.4 GHz) and bridge exactly to the
    first (W, u) pair landing at ~10.4us; the PE never idles, so the gate
    flips once and stays open.
  - The packed wu0 pairs issue on the SP HWDGE ring throttled to <=4 in
    flight (dep chains): unthrottled, the HWDGE lanes round-robin all
    transfers at packet granularity and the pairs complete together
    (~16us) instead of in order; fewer in flight starves bandwidth.  The
    u1 bulk rides the ACT ring gated behind the pairs.  Output stores go
    back on SP.
  - Batch tiles 0 and 1 run K-outer over ib 0-5 (so they only need W/u
    blocks in arrival order) then jb-outer over ib 6-7 so the 8 group
    completions stagger their drains.  Tiles 2-3 run plain jb-outer.
  - All drains run on DVE (PSUM -> SBUF fp16 + per-partition bias add);
    ScalarE is unused, which avoids its on-demand ACT_TABLE_LOAD DMA.
  - PSUM banks and output tiles are pinned per jb (per-tag slots) so a
    batch-tile boundary only waits on its own jb's drain.
  - The final group (bt3, jb7) accumulates as N=448 + N=64 groups in
    separate banks: the 448 part drains + stores while the last 8 matmuls
    run, so the post-stream tail is one [128,64] drain + one 16 KiB store
    (issued on the otherwise-idle ACT ring) + receipt.
"""

import numpy as np

BATCH, DIN, DSTATE, DOUT = 16384, 1024, 2048, 1024
N_CORES = 8
B_SHARD = BATCH // N_CORES  # 2048 rows per core
P = 128                     # SBUF partitions
NB = 512                    # batch tile (moving free dim, max 512 per PSUM bank)
I_BLOCKS = DIN // P         # 8 contraction blocks
J_BLOCKS = DOUT // P        # 8 output-row blocks
B_TILES = B_SHARD // NB     # 4 batch tiles per core
N_WARM = 31                 # N=128 PE warm-up matmuls (HAM clock-gate release)
K_SPLIT = 6                 # hybrid tiles: ib 0..5 K-outer, ib 6..7 jb-outer

_CACHE = {}


def _build_nc():
    import concourse.mybir as mybir
    import concourse.tile as tile
    from concourse.tile import add_dep_helper
    from concourse import bacc
    from concourse._compat import get_trn_type

    nc = bacc.Bacc(get_trn_type() or "TRN2", target_bir_lowering=False)
    f32 = mybir.dt.float32
    f16 = mybir.dt.float16

    wu0 = nc.declare_dram_parameter("wu0", [I_BLOCKS, P, DOUT + NB], f16,
                                    isOutput=False)
    u1 = nc.declare_dram_parameter("u1", [I_BLOCKS, P, 3 * NB], f16, isOutput=False)
    bias = nc.declare_dram_parameter("bias", [P, J_BLOCKS], f32, isOutput=False)
    yb = nc.declare_dram_parameter("yb", [J_BLOCKS, P, B_SHARD], f16, isOutput=True)

    with tile.TileContext(nc) as tc:
        with (
            tc.tile_pool(name="consts", bufs=1) as consts,
            tc.tile_pool(name="opool", bufs=2) as opool,
            tc.tile_pool(name="psum6", bufs=1, space="PSUM") as psum6,
            tc.tile_pool(name="psum2", bufs=2, space="PSUM") as psum2,
        ):
            # PE warm-up junk matmuls: memset on the otherwise-idle GpSimd
            # engine (DVE has framework work first), so the PE starts the
            # HAM busy window at the first possible cycle.
            warm_w = consts.tile([P, P], f16, tag="warm_w")
            nc.gpsimd.memset(warm_w[:], 0.0)
            warm_p = psum2.tile([P, NB], f32, tag="pt67", name="warm_p")
            for _ in range(N_WARM):
                nc.tensor.matmul(warm_p[:, :P], warm_w[:], warm_w[:],
                                 start=True, stop=True)

            # Prefill, serial on the SP ring in consumption order.  Each
            # wu0 row is one DMA (one receipt) delivering the complete
            # (stationary W block, moving u block) pair contraction block ib
            # needs, so the PE unblocks on 8 transfers instead of 16.
            wu_tiles = []
            wu_dmas = []
            for ib in range(I_BLOCKS):
                wu_t = consts.tile([P, DOUT + NB], f16, tag=f"wu{ib}",
                                   name=f"wu{ib}")
                di = nc.sync.dma_start(out=wu_t[:], in_=wu0[ib])
                if ib >= 4:
                    # Throttle to <=4 in flight: unthrottled, the HWDGE lanes
                    # round-robin all transfers at packet granularity and the
                    # pairs complete together (~16us) instead of in order;
                    # fewer than 4 in flight starves bandwidth (one 384 KiB
                    # DMA only sustains ~half the HBM rate).
                    add_dep_helper(di.ins, wu_dmas[ib - 4].ins, sync=True,
                                   reason="prefill in-flight throttle")
                wu_dmas.append(di)
                wu_tiles.append(wu_t)

            bias_t = consts.tile([P, J_BLOCKS], f32, tag="bias")
            bias_dma = nc.scalar.dma_start(out=bias_t[:], in_=bias[:])
            # bias is tiny and first needed at ~20us (first drain); gate it
            # behind wu0 so its packets don't contend during the pair-0
            # window that sets the real-stream start.
            add_dep_helper(bias_dma.ins, wu_dmas[0].ins, sync=True,
                           reason="bias gated out of pair-0 window")
            # u1 bulk rides the ACT ring (SP's FIFO stays free for output
            # stores), gated behind the prefill pairs so it cannot steal HBM
            # bandwidth from them, then self-throttled to <=2 in flight.
            u1_tiles = []
            u1_dmas = []
            for ib in range(I_BLOCKS):
                u_t = consts.tile([P, 3 * NB], f16, tag=f"u1_{ib}",
                                  name=f"u1_{ib}")
                di = nc.scalar.dma_start(out=u_t[:], in_=u1[ib])
                gate = wu_dmas[min(ib + 5, I_BLOCKS - 1)] if ib < 4 else u1_dmas[ib - 4]
                add_dep_helper(di.ins, gate.ins, sync=True,
                               reason="u1 bulk gated behind prefill")
                u1_dmas.append(di)
                u1_tiles.append(u_t)

            def new_pt(jb, bt):
                if jb < 6:
                    return psum6.tile([P, NB], f32, tag=f"pt{jb}",
                                      name=f"pt_{bt}_{jb}")
                return psum2.tile([P, NB], f32, tag="pt67",
                                  name=f"pt_{bt}_{jb}")

            def drain(jb, bt, pt):
                """PSUM -> SBUF fp16 with per-partition bias add, then store."""
                ot = opool.tile([P, NB], f16, tag=f"ot{jb}", name=f"ot_{bt}_{jb}")
                nc.vector.tensor_scalar_add(ot[:], pt[:], bias_t[:, jb:jb + 1])
                nc.sync.dma_start(out=yb[jb][:, bt * NB:(bt + 1) * NB],
                                  in_=ot[:])

            def rhs(bt, ib):
                if bt == 0:
                    return wu_tiles[ib][:, DOUT:DOUT + NB]
                return u1_tiles[ib][:, (bt - 1) * NB:bt * NB]

            for bt in range(B_TILES):
                last_bt = bt == B_TILES - 1
                if bt <= 1:
                    # Hybrid: K-outer over ib 0..3 (consumes W/u blocks in
                    # DMA arrival order; all 8 PSUM groups in flight), then
                    # jb-outer over ib 4..7 staggering group completion.
                    pts = [new_pt(jb, bt) for jb in range(J_BLOCKS)]
                    for ib in range(K_SPLIT):
                        for jb in range(J_BLOCKS):
                            nc.tensor.matmul(
                                pts[jb][:],
                                wu_tiles[ib][:, jb * P:(jb + 1) * P],
                                rhs(bt, ib),
                                start=(ib == 0),
                                stop=False,
                            )
                    for jb in range(J_BLOCKS):
                        for ib in range(K_SPLIT, I_BLOCKS):
                            nc.tensor.matmul(
                                pts[jb][:],
                                wu_tiles[ib][:, jb * P:(jb + 1) * P],
                                rhs(bt, ib),
                                start=False,
                                stop=(ib == I_BLOCKS - 1),
                            )
                        drain(jb, bt, pts[jb])
                    continue
                # jb-outer: drains spread across the batch tile.
                for jb in range(J_BLOCKS):
                    if last_bt and jb == J_BLOCKS - 1:
                        break
                    pt = new_pt(jb, bt)
                    for ib in range(I_BLOCKS):
                        nc.tensor.matmul(
                            pt[:],
                            wu_tiles[ib][:, jb * P:(jb + 1) * P],
                            rhs(bt, ib),
                            start=(ib == 0),
                            stop=(ib == I_BLOCKS - 1),
                        )
                    drain(jb, bt, pt)

            # Final group (bt3, jb7): two N=256 half-groups in separate banks
            # so half the output drains + stores while the last 8 MMs run.
            bt = B_TILES - 1
            jb = J_BLOCKS - 1
            na = 7 * NB // 8          # 448: drains while the last 8 MMs run
            sla = slice((bt - 1) * NB, (bt - 1) * NB + na)
            slb = slice((bt - 1) * NB + na, bt * NB)
            wcol = slice(jb * P, (jb + 1) * P)
            b7 = bias_t[:, jb:jb + 1]
            pta = psum2.tile([P, na], f32, tag="pt67", name="pt_fin_a")
            for ib in range(I_BLOCKS):
                nc.tensor.matmul(pta[:], wu_tiles[ib][:, wcol],
                                 u1_tiles[ib][:, sla],
                                 start=(ib == 0), stop=(ib == I_BLOCKS - 1))
            ptb = psum2.tile([P, NB - na], f32, tag="pt67", name="pt_fin_b")
            for ib in range(I_BLOCKS):
                nc.tensor.matmul(ptb[:], wu_tiles[ib][:, wcol],
                                 u1_tiles[ib][:, slb],
                                 start=(ib == 0), stop=(ib == I_BLOCKS - 1))
            ota = opool.tile([P, na], f16, tag="ot_fa", name="ot_fin_a")
            nc.vector.tensor_scalar_add(ota[:], pta[:], b7)
            nc.sync.dma_start(
                out=yb[jb][:, bt * NB:bt * NB + na], in_=ota[:])
            otb = opool.tile([P, NB - na], f16, tag="ot_fb", name="ot_fin_b")
            nc.vector.tensor_scalar_add(otb[:], ptb[:], b7)
            # the ACT ring is idle by now; SP is still busy with ota's store
            nc.scalar.dma_start(
                out=yb[jb][:, bt * NB + na:(bt + 1) * NB], in_=otb[:])
    nc.compile()
    return nc


def _fold_params(x_re, x_im, nu_log, theta_log, gamma_log, B_re, B_im, C_re, C_im, D):
    """Fold the LRU parameters into (W^T [DIN, DOUT], bias [DOUT]) in float64."""
    nu = np.asarray(nu_log, np.float64)
    th = np.exp(np.asarray(theta_log, np.float64))
    lam_mod = np.exp(-np.exp(nu))
    lam_re = lam_mod * np.cos(th)
    lam_im = lam_mod * np.sin(th)
    g = np.exp(np.asarray(gamma_log, np.float64))
    C_re64 = np.asarray(C_re, np.float64)
    C_im64 = np.asarray(C_im, np.float64)
    W = (2.0 * ((C_re64 * g) @ np.asarray(B_re, np.float64))
         - 2.0 * ((C_im64 * g) @ np.asarray(B_im, np.float64))
         + np.asarray(D, np.float64))  # [DOUT, DIN]
    xr = np.asarray(x_re, np.float64)
    xi = np.asarray(x_im, np.float64)
    lx_re = lam_re * xr - lam_im * xi
    lx_im = lam_re * xi + lam_im * xr
    bias = 2.0 * (C_re64 @ lx_re - C_im64 @ lx_im)  # [DOUT]
    return W.T.astype(np.float16), bias.astype(np.float32)


def kernel(u_in, x_re, x_im, nu_log, theta_log, gamma_log, B_re, B_im,
           C_re, C_im, D, _trace=False):
    from concourse.bass_utils import run_bass_kernel_spmd

    wt16, bias_host = _fold_params(
        x_re, x_im, nu_log, theta_log, gamma_log, B_re, B_im, C_re, C_im, D)
    wtb = wt16.reshape(I_BLOCKS, P, DOUT)
    bias2 = np.ascontiguousarray(bias_host.reshape(J_BLOCKS, P).T)  # [128, 8]

    u2 = np.asarray(u_in, np.float32).reshape(BATCH, DIN).astype(np.float16)
    core_ids = list(range(N_CORES))
    in_maps = []
    for c in core_ids:
        # uc[ib, p, b] = shard[b, ib*P + p]
        uc = np.ascontiguousarray(
            u2[c * B_SHARD:(c + 1) * B_SHARD].T).reshape(I_BLOCKS, P, B_SHARD)
        in_maps.append({
            "wu0": np.ascontiguousarray(
                np.concatenate((wtb, uc[:, :, :NB]), axis=2)),
            "u1": np.ascontiguousarray(uc[:, :, NB:]),
            "bias": bias2,
        })

    if "nc" not in _CACHE:
        _CACHE["nc"] = _build_nc()
    res = run_bass_kernel_spmd(_CACHE["nc"], in_maps, core_ids, trace=_trace)

    y = np.empty((BATCH, DOUT), np.float32)
    for c in core_ids:
        # yb[jb, p, b] = y_shard[b, jb*P + p]
        ybc = res.results[c]["yb"]
        y[c * B_SHARD:(c + 1) * B_SHARD] = ybc.reshape(DOUT, B_SHARD).T
    out = y.reshape(BATCH, 1, DOUT)
    if _trace:
        return out, res
    return out
